# revision 11
# baseline (speedup 1.0000x reference)
"""Bahdanau attention Trainium2 Bass kernel (bf16 PE pipeline).

Problem (fixed shapes):
  decoder_state [32, 1024] f32, encoder_hiddens [32, 2048, 1024] f32,
  Wa_w [1,1024], Wa_b [1], Wb_w [1024,1024], Wb_b [1024], Wc_w [1024,1024], Wc_b [1024]
  out: context [32, 1024] f32

Strategy: data-parallel over batch, 4 batches per core on 8 cores.
encoder_hiddens is cast f32->bf16 inside the SWDGE load; on-chip PE
transposes (bf16, batched per block to limit matmul/transpose mode-switch
interference) produce [h,s] tiles; the PE then streams bf16 matmuls at the
1 col/cycle roofline (~216ns per 128x128x512 MM). The score reduction
sum_k wa_k*e runs off the PE: a DVE scalar_tensor_tensor chain scales each
k-tile by wa (per-partition scalar) and a GpSimd partition_all_reduce
finishes the cross-partition sum. Softmax is flash-style per 512-wide
s-block; context partials accumulate on the DVE; per-batch combines are
deferred to a short PE tail.
"""
import sys

if "/opt/trn_rl_repo" not in sys.path:
    sys.path.insert(0, "/opt/trn_rl_repo")

import numpy as np

import concourse.bass as bass
import concourse.tile as tile
from concourse import bacc, mybir
from concourse import bass_utils
from concourse.masks import make_identity
from bass_rust import ReduceOp

F32 = mybir.dt.float32
F32R = mybir.dt.float32r
BF16 = mybir.dt.bfloat16

B, S, H, K = 32, 2048, 1024, 1024
NCORES = 8
BLOC = B // NCORES          # batches per core
SBLK = 512                  # s-block (softmax block, PE moving width)
NBLK = S // SBLK            # 4
NST = SBLK // 128           # s-tiles per block: 4
NHT = H // 128              # 8
NKT = K // 128              # 8


def build_kernel():
    nc = bacc.Bacc("TRN2", target_bir_lowering=False)

    enc = nc.dram_tensor("enc", [BLOC, S, H], F32, kind="ExternalInput")
    dec = nc.dram_tensor("dec", [BLOC, H], F32, kind="ExternalInput")
    wa = nc.dram_tensor("wa", [1, K], F32, kind="ExternalInput")
    wb = nc.dram_tensor("wb", [K, H], F32, kind="ExternalInput")
    wbb = nc.dram_tensor("wbb", [1, K], F32, kind="ExternalInput")
    wc = nc.dram_tensor("wc", [K, H], F32, kind="ExternalInput")
    wcb = nc.dram_tensor("wcb", [1, K], F32, kind="ExternalInput")
    y = nc.dram_tensor("y", [BLOC, H], F32, kind="ExternalOutput")

    TT = mybir.ActivationFunctionType.Tanh
    EX = mybir.ActivationFunctionType.Exp
    CP = mybir.ActivationFunctionType.Copy
    ADD = mybir.AluOpType.add
    MULT = mybir.AluOpType.mult

    from contextlib import ExitStack
    with tile.TileContext(nc) as tc, ExitStack() as stack:
        consts = stack.enter_context(tc.tile_pool(name="consts", bufs=1))
        identf = consts.tile([128, 128], F32)
        make_identity(nc, identf)
        identb = consts.tile([128, 128], BF16)
        nc.vector.tensor_copy(identb, identf)
        wcT = consts.tile([128, NHT, K], BF16)      # [h, ht, k]
        waT = consts.tile([128, NKT], BF16)
        bias_kb = consts.tile([128, NKT, BLOC], F32)

        # --- enc streaming pools ---
        enc_p = stack.enter_context(tc.tile_pool(name="enc_nat", bufs=2))
        encT_p = stack.enter_context(tc.tile_pool(name="encT", bufs=4))
        ps_tr = stack.enter_context(tc.tile_pool(name="ps_tr", bufs=3, space="PSUM"))

        def load_enc(b, blk):
            # SWDGE cast-load f32 -> bf16, [s_p=128, st, h]
            t = enc_p.tile([128, NST, H], BF16, tag="en")
            half = NST // 2
            for hh in range(2):
                nc.gpsimd.dma_start(
                    out=t[:, hh * half:(hh + 1) * half, :],
                    in_=enc[b, blk * SBLK + hh * half * 128:
                            blk * SBLK + (hh + 1) * half * 128, :].rearrange(
                        "(st sp) h -> sp st h", sp=128))
            return t

        def transpose_block(enc_nat):
            # PE transposes (bf16): [s, h] -> [h, s] tiles, one burst per block
            eT = encT_p.tile([128, NHT, SBLK], BF16, tag="eT")
            for ht in range(NHT):
                pst = ps_tr.tile([128, SBLK], BF16, tag="tp")
                for st in range(NST):
                    nc.tensor.transpose(pst[:, st * 128:(st + 1) * 128],
                                        enc_nat[:, st, ht * 128:(ht + 1) * 128],
                                        identb)
                if ht % 2 == 0:
                    nc.vector.tensor_copy(eT[:, ht, :], pst)
                else:
                    nc.scalar.activation(eT[:, ht, :], pst, CP)
            return eT

        # ---------------- setup ----------------
        with tc.tile_pool(name="setup", bufs=1) as setup, \
             tc.tile_pool(name="setup_ps", bufs=1, space="PSUM") as sps:
            # SWDGE bf16 cast-loads: wc (feeds main MMs), wa
            wc_nat = setup.tile([128, NKT, H], BF16, tag="wc_nat")
            nc.gpsimd.dma_start(
                out=wc_nat, in_=wc.rearrange("(kt kp) h -> kp kt h", kp=128))
            wb_nat = setup.tile([128, NKT, H], BF16, tag="wb_nat")
            nc.gpsimd.dma_start(
                out=wb_nat, in_=wb.rearrange("(kt kp) h -> kp kt h", kp=128))

            # enc prefetch for blocks (0,0), (0,1) behind wc/wb on SWDGE
            pre_nat = {}
            for bb in ((0, 0), (0, 1)):
                pre_nat[bb] = load_enc(*bb)
            wa_nat = setup.tile([1, K], BF16, tag="wa_nat")
            nc.gpsimd.dma_start(out=wa_nat, in_=wa[:, :])

            # HWDGE f32 loads (parallel queue-rings with SWDGE)
            dec_nat = setup.tile([BLOC, H], F32)
            nc.sync.dma_start(out=dec_nat, in_=dec[:, :])
            wbb_r = setup.tile([1, K], F32, tag="brin")
            wcb_r = setup.tile([1, K], F32, tag="brin2")
            nc.sync.dma_start(out=wbb_r, in_=wbb[:, :])
            nc.sync.dma_start(out=wcb_r, in_=wcb[:, :])

            # wcT via PE transposes (bf16)
            for ht in range(NHT):
                for ktg in range(2):
                    psc = ps_tr.tile([128, 512], BF16, tag="tp")
                    for kq in range(NKT // 2):
                        kt = ktg * 4 + kq
                        nc.tensor.transpose(
                            psc[:, kq * 128:(kq + 1) * 128],
                            wc_nat[:, kt, ht * 128:(ht + 1) * 128], identb)
                    nc.scalar.activation(
                        wcT[:, ht, ktg * 512:(ktg + 1) * 512], psc, CP)

            # decT via PE transposes (f32), cast to bf16 for dec_proj MMs
            decT = setup.tile([128, NHT, BLOC], BF16)
            for ht in range(NHT):
                ps = sps.tile([128, BLOC], F32, tag="tp")
                nc.tensor.transpose(ps, dec_nat[:, ht * 128:(ht + 1) * 128],
                                    identf[0:BLOC, 0:BLOC])
                nc.vector.tensor_copy(decT[:, ht, :], ps)

            # wbT via PE transposes (bf16), PSUM -> SBUF via ps_tr ring
            wbT = setup.tile([128, NHT, K], BF16, tag="wbT")   # [h, ht, k]
            for ht in range(NHT):
                for ktg in range(2):
                    psb = ps_tr.tile([128, 512], BF16, tag="tp")
                    for kq in range(NKT // 2):
                        kt = ktg * 4 + kq
                        nc.tensor.transpose(
                            psb[:, kq * 128:(kq + 1) * 128],
                            wb_nat[:, kt, ht * 128:(ht + 1) * 128], identb)
                    nc.scalar.activation(
                        wbT[:, ht, ktg * 512:(ktg + 1) * 512], psb, CP)

            # waT via PE transposes (bf16)
            for kt in range(NKT):
                psw = sps.tile([128, 1], BF16, tag="tw")
                nc.tensor.transpose(psw, wa_nat[:, kt * 128:(kt + 1) * 128],
                                    identb[0:1, 0:1])
                nc.vector.tensor_copy(waT[:, kt:kt + 1], psw)

            # transpose the two prefetched enc blocks (their DMA has landed by
            # the time the weight transposes above are done)
            pre = {}
            for bb in ((0, 0), (0, 1)):
                pre[bb] = transpose_block(pre_nat[bb])

            # dec_proj[b, k] = sum_h decT[h,b].T @ wbT[h,k]
            dproj = setup.tile([BLOC, K], F32)
            for kh in range(2):
                psd = sps.tile([BLOC, 512], F32, tag="pd")
                for ht in range(NHT):
                    nc.tensor.matmul(psd, decT[:, ht, :],
                                     wbT[:, ht, kh * 512:(kh + 1) * 512],
                                     start=(ht == 0), stop=(ht == NHT - 1))
                nc.vector.tensor_copy(dproj[:, kh * 512:(kh + 1) * 512], psd)

            # bias row: Wb_b + Wc_b -> [k,1] segments
            brow = setup.tile([1, K], F32)
            nc.vector.tensor_tensor(out=brow, in0=wbb_r, in1=wcb_r, op=ADD)
            bseg = setup.tile([128, NKT], F32)
            for kt in range(NKT):
                ps1 = sps.tile([128, BLOC], F32, tag="tp")
                nc.tensor.transpose(ps1[:, 0:1], brow[:, kt * 128:(kt + 1) * 128],
                                    identf[0:1, 0:1])
                nc.vector.tensor_copy(bseg[:, kt:kt + 1], ps1[:, 0:1])

            # bias_kb[:, kt, b] = dproj^T + bseg
            for kt in range(NKT):
                ps2 = sps.tile([128, BLOC], F32, tag="tp")
                nc.tensor.transpose(ps2, dproj[:, kt * 128:(kt + 1) * 128],
                                    identf[0:BLOC, 0:BLOC])
                bs = bseg[:, kt:kt + 1]
                nc.vector.tensor_tensor(
                    out=bias_kb[:, kt, :], in0=ps2,
                    in1=bass.AP(tensor=bs.tensor, offset=bs.offset,
                                ap=[bs.ap[0], [0, BLOC]]),
                    op=ADD)

        # ---------------- main loop ----------------
        e_p = stack.enter_context(tc.tile_pool(name="e", bufs=18))
        row_p = stack.enter_context(tc.tile_pool(name="rows", bufs=3))
        stat_p = stack.enter_context(tc.tile_pool(name="stats", bufs=2))
        ctxT_p = stack.enter_context(tc.tile_pool(name="ctxT", bufs=10))
        acc_p = stack.enter_context(tc.tile_pool(name="acc", bufs=BLOC))
        bc_p = stack.enter_context(tc.tile_pool(name="bcast", bufs=3))
        ps_e = stack.enter_context(tc.tile_pool(name="ps_e", bufs=3, space="PSUM"))
        ps_s = stack.enter_context(tc.tile_pool(name="ps_s", bufs=1, space="PSUM"))
        ps_o = stack.enter_context(tc.tile_pool(name="ps_o", bufs=1, space="PSUM"))

        def do_scores_softmax_context(task):
            b, blk, encT, e_sb, mrow, zrow, ctxT_blks = task
            # scores: PE matmuls (deferred one block, et is long ready)
            pss = ps_s.tile([1, SBLK], F32, tag="sc")
            for kt in range(NKT):
                nc.tensor.matmul(pss, waT[:, kt:kt + 1], e_sb[kt],
                                 start=(kt == 0), stop=(kt == NKT - 1))

            # block softmax straight from PSUM: m_blk, w = exp(s-m) (bf16), Z
            negm = row_p.tile([1, 1], F32, tag="negm")
            nc.vector.reduce_max(negm, pss, axis=mybir.AxisListType.X,
                                 negate=True)
            wrow = row_p.tile([1, SBLK], BF16, tag="wrow")
            nc.scalar.activation(wrow, pss, EX, bias=negm,
                                 accum_out=zrow[:, blk:blk + 1])
            nc.vector.tensor_scalar_mul(mrow[:, blk:blk + 1], negm, -1.0)

            # context partial on DVE (bf16 inputs, f32 accum)
            wb_t = bc_p.tile([128, SBLK], BF16, tag="wb")
            nc.gpsimd.partition_broadcast(wb_t, wrow, 128)
            ctxT_blk = ctxT_p.tile([128, NHT], F32, tag="ct")
            for ht in range(NHT):
                scr = bc_p.tile([128, SBLK], BF16, tag="scr")
                nc.vector.scalar_tensor_tensor(
                    out=scr, in0=encT[:, ht, :], scalar=1.0, in1=wb_t,
                    op0=MULT, op1=MULT, accum_out=ctxT_blk[:, ht:ht + 1])
            ctxT_blks.append(ctxT_blk)

        def do_combine(task):
            # batch combine on DVE: acc = sum_blk C_blk * exp(m_blk - m_g) / Z
            b, mrow, zrow, ctxT_blks = task
            negmg = stat_p.tile([1, 1], F32, tag="negmg")
            nc.vector.reduce_max(negmg, mrow, axis=mybir.AxisListType.X,
                                 negate=True)
            fb = stat_p.tile([1, NBLK], F32, tag="fb")
            nc.scalar.activation(fb, mrow, EX, bias=negmg)
            zf = stat_p.tile([1, NBLK], F32, tag="zf")
            nc.gpsimd.tensor_tensor(out=zf, in0=zrow, in1=fb, op=MULT)
            z = stat_p.tile([1, 1], F32, tag="z")
            nc.vector.reduce_sum(z, zf, axis=mybir.AxisListType.X)
            rz = stat_p.tile([1, 1], F32, tag="rz")
            nc.vector.reciprocal(rz, z)
            frow = stat_p.tile([1, NBLK], F32, tag="frow")
            nc.vector.tensor_scalar_mul(frow, fb, rz)
            fB = stat_p.tile([128, NBLK], F32, tag="fB")
            nc.gpsimd.partition_broadcast(fB, frow, 128)

            acc = acc_p.tile([128, NHT], F32, tag=f"acc{b}")
            nc.vector.tensor_scalar_mul(acc, ctxT_blks[0], fB[:, 0:1])
            for blk in range(1, NBLK):
                nc.vector.scalar_tensor_tensor(
                    out=acc, in0=ctxT_blks[blk], scalar=fB[:, blk:blk + 1],
                    in1=acc, op0=MULT, op1=ADD)
            accs.append(acc)

        accs = []
        pending = None
        pending_fin = None
        cur_nat = None
        for b in range(BLOC):
            mrow = stat_p.tile([1, NBLK], F32, tag="mrow")
            zrow = stat_p.tile([1, NBLK], F32, tag="zrow")
            ctxT_blks = []
            for blk in range(NBLK):
                nxt = (b, blk + 1) if blk + 1 < NBLK else (b + 1, 0)
                if (b, blk) in pre:
                    encT = pre.pop((b, blk))
                else:
                    encT = cur
                if nxt[0] < BLOC and nxt not in pre:
                    nxt_nat = load_enc(*nxt)
                else:
                    nxt_nat = None

                # enc_proj (k-tiles) + tanh -> e (bf16)
                e_sb = []
                for kt in range(NKT):
                    pse = ps_e.tile([128, SBLK], F32, tag="pe")
                    for ht in range(NHT):
                        nc.tensor.matmul(pse, wcT[:, ht, kt * 128:(kt + 1) * 128],
                                         encT[:, ht, :],
                                         start=(ht == 0), stop=(ht == NHT - 1))
                    et = e_p.tile([128, SBLK], BF16, tag="et")
                    nc.scalar.activation(et, pse, TT, bias=bias_kb[:, kt, b:b + 1])
                    e_sb.append(et)

                # deferred non-PE tail of the previous block, then its combine
                if pending is not None:
                    do_scores_softmax_context(pending)
                    pending = None
                if pending_fin is not None:
                    do_combine(pending_fin)
                    pending_fin = None
                pending = (b, blk, encT, e_sb, mrow, zrow, ctxT_blks)
                if blk == NBLK - 1:
                    pending_fin = (b, mrow, zrow, ctxT_blks)

                # transpose the next block at the section end (its DMA had a
                # full section to land)
                if nxt_nat is not None:
                    cur = transpose_block(nxt_nat)

        do_scores_softmax_context(pending)
        do_combine(pending_fin)

        # ---------------- tail: transpose accs to rows, store ----------------
        # identf_late is written by the DVE only after the last combine's DVE
        # ops (engine streams are in-order), pinning the tail transposes to
        # the end of the PE stream regardless of scheduler hoisting.
        identf_late = consts.tile([128, 128], F32)
        nc.vector.tensor_copy(identf_late, identf)
        for b in range(BLOC):
            acc = accs[b]
            ctx_row = stat_p.tile([1, H], F32, tag=f"ctxr{b % 2}")
            for hb in range(2):
                pso = ps_o.tile([1, 512], F32, tag="or")
                for hq in range(NHT // 2):
                    ht = hb * (NHT // 2) + hq
                    nc.tensor.transpose(pso[:, hq * 128:(hq + 1) * 128],
                                        acc[:, ht:ht + 1], identf_late)
                nc.vector.tensor_copy(ctx_row[:, hb * 512:(hb + 1) * 512], pso)
            nc.sync.dma_start(out=y[b:b + 1, :], in_=ctx_row)

    nc.compile()
    return nc


_NC_CACHE = None


def _get_nc():
    global _NC_CACHE
    if _NC_CACHE is None:
        _NC_CACHE = build_kernel()
    return _NC_CACHE


def kernel(decoder_state, encoder_hiddens, Wa_w, Wa_b, Wb_w, Wb_b, Wc_w, Wc_b,
           **run_kwargs):
    decoder_state = np.ascontiguousarray(decoder_state, dtype=np.float32)
    encoder_hiddens = np.ascontiguousarray(encoder_hiddens, dtype=np.float32)
    nc = _get_nc()
    in_maps = []
    for c in range(NCORES):
        in_maps.append({
            "enc": encoder_hiddens[c * BLOC:(c + 1) * BLOC],
            "dec": decoder_state[c * BLOC:(c + 1) * BLOC],
            "wa": np.ascontiguousarray(Wa_w, dtype=np.float32).reshape(1, K),
            "wb": np.ascontiguousarray(Wb_w, dtype=np.float32),
            "wbb": np.ascontiguousarray(Wb_b, dtype=np.float32).reshape(1, K),
            "wc": np.ascontiguousarray(Wc_w, dtype=np.float32),
            "wcb": np.ascontiguousarray(Wc_b, dtype=np.float32).reshape(1, K),
        })
    res = bass_utils.run_bass_kernel_spmd(
        nc, in_maps, core_ids=list(range(NCORES)), **run_kwargs)
    out = np.concatenate([res.results[c]["y"] for c in range(NCORES)], axis=0)
    # Wa_b shifts every score equally; softmax is invariant to it.
    if run_kwargs:
        return out, res
    return out


# revision 13
# speedup vs baseline: 1.1179x; 1.1179x over previous
"""Bahdanau attention Trainium2 Bass kernel (bf16 PE pipeline).

Problem (fixed shapes):
  decoder_state [32, 1024] f32, encoder_hiddens [32, 2048, 1024] f32,
  Wa_w [1,1024], Wa_b [1], Wb_w [1024,1024], Wb_b [1024], Wc_w [1024,1024], Wc_b [1024]
  out: context [32, 1024] f32

Strategy: data-parallel over batch, 4 batches per core on 8 cores.
encoder_hiddens is cast f32->bf16 inside the SWDGE load; on-chip PE
transposes (bf16, one burst per block) produce [h,s] tiles; the PE streams
bf16 matmuls at the 1 col/cycle roofline (~216ns per 128x128x512 MM).
Scores run as PE matmuls deferred one block (et long ready -> no stall).
Softmax is flash-style per 512-wide s-block; context partials accumulate
on the DVE in [h,ht] column form; per-batch combines transpose to rows on
the PE. Emission order per section keeps each engine FIFO free of
head-of-line blocking: MMs+tanh, next-block transposes, then deferred
score/softmax/context work. The kernel-final block computes its context
directly in row form on the PE to shorten the serial tail.
"""
import sys

if "/opt/trn_rl_repo" not in sys.path:
    sys.path.insert(0, "/opt/trn_rl_repo")

import numpy as np

import concourse.bass as bass
import concourse.tile as tile
from concourse import bacc, mybir
from concourse import bass_utils
from concourse.masks import make_identity

F32 = mybir.dt.float32
F32R = mybir.dt.float32r
BF16 = mybir.dt.bfloat16

B, S, H, K = 32, 2048, 1024, 1024
NCORES = 8
BLOC = B // NCORES          # batches per core
SBLK = 512                  # s-block (softmax block, PE moving width)
NBLK = S // SBLK            # 4
NST = SBLK // 128           # s-tiles per block: 4
NHT = H // 128              # 8
NKT = K // 128              # 8


def build_kernel():
    nc = bacc.Bacc("TRN2", target_bir_lowering=False)

    enc = nc.dram_tensor("enc", [BLOC, S, H], F32, kind="ExternalInput")
    dec = nc.dram_tensor("dec", [BLOC, H], F32, kind="ExternalInput")
    wa = nc.dram_tensor("wa", [1, K], F32, kind="ExternalInput")
    wb = nc.dram_tensor("wb", [K, H], F32, kind="ExternalInput")
    wbb = nc.dram_tensor("wbb", [1, K], F32, kind="ExternalInput")
    wc = nc.dram_tensor("wc", [K, H], F32, kind="ExternalInput")
    wcb = nc.dram_tensor("wcb", [1, K], F32, kind="ExternalInput")
    y = nc.dram_tensor("y", [BLOC, H], F32, kind="ExternalOutput")

    TT = mybir.ActivationFunctionType.Tanh
    EX = mybir.ActivationFunctionType.Exp
    CP = mybir.ActivationFunctionType.Copy
    ADD = mybir.AluOpType.add
    MULT = mybir.AluOpType.mult

    from contextlib import ExitStack
    with tile.TileContext(nc) as tc, ExitStack() as stack:
        consts = stack.enter_context(tc.tile_pool(name="consts", bufs=1))
        identf = consts.tile([128, 128], F32)
        make_identity(nc, identf)
        identb = consts.tile([128, 128], BF16)
        nc.vector.tensor_copy(identb, identf)
        wcT = consts.tile([128, NHT, K], BF16)      # [h, ht, k]
        waT = consts.tile([128, NKT], BF16)
        bias_kb = consts.tile([128, NKT, BLOC], F32)

        # --- enc streaming pools ---
        enc_p = stack.enter_context(tc.tile_pool(name="enc_nat", bufs=2))
        encT_p = stack.enter_context(tc.tile_pool(name="encT", bufs=4))
        ps_tr = stack.enter_context(tc.tile_pool(name="ps_tr", bufs=3, space="PSUM"))

        def load_enc(b, blk):
            # SWDGE cast-load f32 -> bf16, [s_p=128, st, h]
            t = enc_p.tile([128, NST, H], BF16, tag="en")
            half = NST // 2
            for hh in range(2):
                nc.gpsimd.dma_start(
                    out=t[:, hh * half:(hh + 1) * half, :],
                    in_=enc[b, blk * SBLK + hh * half * 128:
                            blk * SBLK + (hh + 1) * half * 128, :].rearrange(
                        "(st sp) h -> sp st h", sp=128))
            return t

        def transpose_block(enc_nat):
            # PE transposes (bf16): [s, h] -> [h, s] tiles, one burst per block
            eT = encT_p.tile([128, NHT, SBLK], BF16, tag="eT")
            for ht in range(NHT):
                pst = ps_tr.tile([128, SBLK], BF16, tag="tp")
                for st in range(NST):
                    nc.tensor.transpose(pst[:, st * 128:(st + 1) * 128],
                                        enc_nat[:, st, ht * 128:(ht + 1) * 128],
                                        identb)
                nc.vector.tensor_copy(eT[:, ht, :], pst)
            return eT

        # ---------------- setup ----------------
        with tc.tile_pool(name="setup", bufs=1) as setup, \
             tc.tile_pool(name="setup_ps", bufs=1, space="PSUM") as sps:
            # SWDGE bf16 cast-loads, in dependency order: wc, wb, wa, enc
            wc_nat = setup.tile([128, NKT, H], BF16, tag="wc_nat")
            nc.gpsimd.dma_start(
                out=wc_nat, in_=wc.rearrange("(kt kp) h -> kp kt h", kp=128))
            wb_nat = setup.tile([128, NKT, H], BF16, tag="wb_nat")
            nc.gpsimd.dma_start(
                out=wb_nat, in_=wb.rearrange("(kt kp) h -> kp kt h", kp=128))
            wa_nat = setup.tile([1, K], BF16, tag="wa_nat")
            nc.gpsimd.dma_start(out=wa_nat, in_=wa[:, :])
            pre_nat = {}
            for bb in ((0, 0), (0, 1)):
                pre_nat[bb] = load_enc(*bb)

            # HWDGE f32 loads (parallel queue-rings with SWDGE)
            dec_nat = setup.tile([BLOC, H], F32)
            nc.sync.dma_start(out=dec_nat, in_=dec[:, :])
            wbb_r = setup.tile([1, K], F32, tag="brin")
            wcb_r = setup.tile([1, K], F32, tag="brin2")
            nc.sync.dma_start(out=wbb_r, in_=wbb[:, :])
            nc.sync.dma_start(out=wcb_r, in_=wcb[:, :])

            # wcT via PE transposes (bf16), ps_tr ring
            for ht in range(NHT):
                for ktg in range(2):
                    psc = ps_tr.tile([128, SBLK], BF16, tag="tp")
                    for kq in range(NKT // 2):
                        kt = ktg * 4 + kq
                        nc.tensor.transpose(
                            psc[:, kq * 128:(kq + 1) * 128],
                            wc_nat[:, kt, ht * 128:(ht + 1) * 128], identb)
                    nc.scalar.activation(
                        wcT[:, ht, ktg * 512:(ktg + 1) * 512], psc, CP)

            # wbT via PE transposes (bf16), ps_tr ring
            wbT = setup.tile([128, NHT, K], BF16, tag="wbT")   # [h, ht, k]
            for ht in range(NHT):
                for ktg in range(2):
                    psb = ps_tr.tile([128, SBLK], BF16, tag="tp")
                    for kq in range(NKT // 2):
                        kt = ktg * 4 + kq
                        nc.tensor.transpose(
                            psb[:, kq * 128:(kq + 1) * 128],
                            wb_nat[:, kt, ht * 128:(ht + 1) * 128], identb)
                    nc.scalar.activation(
                        wbT[:, ht, ktg * 512:(ktg + 1) * 512], psb, CP)

            # decT via PE transposes (f32), cast to bf16 for dec_proj MMs
            decT = setup.tile([128, NHT, BLOC], BF16)
            for ht in range(NHT):
                ps = sps.tile([128, BLOC], F32, tag="tp")
                nc.tensor.transpose(ps, dec_nat[:, ht * 128:(ht + 1) * 128],
                                    identf[0:BLOC, 0:BLOC])
                nc.vector.tensor_copy(decT[:, ht, :], ps)

            # waT via PE transposes (bf16)
            for kt in range(NKT):
                psw = sps.tile([128, 1], BF16, tag="tw")
                nc.tensor.transpose(psw, wa_nat[:, kt * 128:(kt + 1) * 128],
                                    identb[0:1, 0:1])
                nc.vector.tensor_copy(waT[:, kt:kt + 1], psw)

            # transpose the two prefetched enc blocks (their DMA has landed by
            # the time the weight transposes above are done)
            pre = {}
            for bb in ((0, 0), (0, 1)):
                pre[bb] = transpose_block(pre_nat[bb])

            # dec_proj[b, k] = sum_h decT[h,b].T @ wbT[h,k]
            dproj = setup.tile([BLOC, K], F32)
            for kh in range(2):
                psd = sps.tile([BLOC, 512], F32, tag="pd")
                for ht in range(NHT):
                    nc.tensor.matmul(psd, decT[:, ht, :],
                                     wbT[:, ht, kh * 512:(kh + 1) * 512],
                                     start=(ht == 0), stop=(ht == NHT - 1))
                nc.vector.tensor_copy(dproj[:, kh * 512:(kh + 1) * 512], psd)

            # bias row: Wb_b + Wc_b -> [k,1] segments
            brow = setup.tile([1, K], F32)
            nc.vector.tensor_tensor(out=brow, in0=wbb_r, in1=wcb_r, op=ADD)
            bseg = setup.tile([128, NKT], F32)
            for kt in range(NKT):
                ps1 = sps.tile([128, BLOC], F32, tag="tp")
                nc.tensor.transpose(ps1[:, 0:1], brow[:, kt * 128:(kt + 1) * 128],
                                    identf[0:1, 0:1])
                nc.vector.tensor_copy(bseg[:, kt:kt + 1], ps1[:, 0:1])

            # bias_kb[:, kt, b] = dproj^T + bseg
            for kt in range(NKT):
                ps2 = sps.tile([128, BLOC], F32, tag="tp")
                nc.tensor.transpose(ps2, dproj[:, kt * 128:(kt + 1) * 128],
                                    identf[0:BLOC, 0:BLOC])
                bs = bseg[:, kt:kt + 1]
                nc.vector.tensor_tensor(
                    out=bias_kb[:, kt, :], in0=ps2,
                    in1=bass.AP(tensor=bs.tensor, offset=bs.offset,
                                ap=[bs.ap[0], [0, BLOC]]),
                    op=ADD)

        # ---------------- main loop ----------------
        e_p = stack.enter_context(tc.tile_pool(name="e", bufs=18))
        row_p = stack.enter_context(tc.tile_pool(name="rows", bufs=3))
        stat_p = stack.enter_context(tc.tile_pool(name="stats", bufs=2))
        ctxT_p = stack.enter_context(tc.tile_pool(name="ctxT", bufs=10))
        acc_p = stack.enter_context(tc.tile_pool(name="acc", bufs=BLOC))
        bc_p = stack.enter_context(tc.tile_pool(name="bcast", bufs=3))
        ps_e = stack.enter_context(tc.tile_pool(name="ps_e", bufs=3, space="PSUM"))
        ps_s = stack.enter_context(tc.tile_pool(name="ps_s", bufs=1, space="PSUM"))
        ps_o = stack.enter_context(tc.tile_pool(name="ps_o", bufs=1, space="PSUM"))

        def do_scores_softmax(task):
            b, blk, encT, e_sb, mrow, zrow, ctxT_blks = task
            # scores: PE matmuls (deferred one block, et is long ready)
            pss = ps_s.tile([1, SBLK], F32, tag="sc")
            for kt in range(NKT):
                nc.tensor.matmul(pss, waT[:, kt:kt + 1], e_sb[kt],
                                 start=(kt == 0), stop=(kt == NKT - 1))

            # block softmax straight from PSUM: m_blk, w = exp(s-m) (bf16), Z
            negm = row_p.tile([1, 1], F32, tag="negm")
            nc.vector.reduce_max(negm, pss, axis=mybir.AxisListType.X,
                                 negate=True)
            wrow = row_p.tile([1, SBLK], BF16, tag="wrow")
            nc.scalar.activation(wrow, pss, EX, bias=negm,
                                 accum_out=zrow[:, blk:blk + 1])
            nc.vector.tensor_scalar_mul(mrow[:, blk:blk + 1], negm, -1.0)
            return wrow

        def do_context(task, wrow):
            b, blk, encT, e_sb, mrow, zrow, ctxT_blks = task
            # context partial on DVE (bf16 inputs, f32 accum)
            wb_t = bc_p.tile([128, SBLK], BF16, tag="wb")
            nc.gpsimd.partition_broadcast(wb_t, wrow, 128)
            ctxT_blk = ctxT_p.tile([128, NHT], F32, tag="ct")
            for ht in range(NHT):
                scr = bc_p.tile([128, SBLK], BF16, tag="scr")
                nc.vector.scalar_tensor_tensor(
                    out=scr, in0=encT[:, ht, :], scalar=1.0, in1=wb_t,
                    op0=MULT, op1=MULT, accum_out=ctxT_blk[:, ht:ht + 1])
            ctxT_blks.append(ctxT_blk)

        def combine_factors(b, mrow, zrow):
            # f[blk] = exp(m_blk - m_g) / Z, broadcast to all partitions
            negmg = stat_p.tile([1, 1], F32, tag="negmg")
            nc.vector.reduce_max(negmg, mrow, axis=mybir.AxisListType.X,
                                 negate=True)
            fb = stat_p.tile([1, NBLK], F32, tag="fb")
            nc.scalar.activation(fb, mrow, EX, bias=negmg)
            zf = stat_p.tile([1, NBLK], F32, tag="zf")
            nc.vector.tensor_tensor(out=zf, in0=zrow, in1=fb, op=MULT)
            z = stat_p.tile([1, 1], F32, tag="z")
            nc.vector.reduce_sum(z, zf, axis=mybir.AxisListType.X)
            rz = stat_p.tile([1, 1], F32, tag="rz")
            nc.vector.reciprocal(rz, z)
            frow = stat_p.tile([1, NBLK], F32, tag="frow")
            nc.vector.tensor_scalar_mul(frow, fb, rz)
            fB = stat_p.tile([128, NBLK], F32, tag="fB")
            nc.gpsimd.partition_broadcast(fB, frow, 128)
            return frow, fB

        def do_combine(task):
            # batch combine on DVE: acc = sum_blk C_blk * exp(m_blk - m_g) / Z
            b, mrow, zrow, ctxT_blks = task
            frow, fB = combine_factors(b, mrow, zrow)
            acc = acc_p.tile([128, NHT], F32, tag=f"acc{b}")
            nc.vector.tensor_scalar_mul(acc, ctxT_blks[0], fB[:, 0:1])
            for blk in range(1, NBLK):
                nc.vector.scalar_tensor_tensor(
                    out=acc, in0=ctxT_blks[blk], scalar=fB[:, blk:blk + 1],
                    in1=acc, op0=MULT, op1=ADD)
            accs.append(acc)

        def emit_tail(b, acc, ident):
            # transpose acc [h,ht] -> ctx_row [1, H], store
            ctx_row = stat_p.tile([1, H], F32, tag=f"ctxr{b % 2}")
            for hb in range(2):
                pso = ps_o.tile([1, 512], F32, tag="or")
                for hq in range(NHT // 2):
                    ht = hb * (NHT // 2) + hq
                    nc.tensor.transpose(pso[:, hq * 128:(hq + 1) * 128],
                                        acc[:, ht:ht + 1], ident)
                nc.vector.tensor_copy(ctx_row[:, hb * 512:(hb + 1) * 512], pso)
            nc.sync.dma_start(out=y[b:b + 1, :], in_=ctx_row)

        accs = []
        pending = None
        pending_fin = None
        cur = None
        last_nat = None
        tails_emitted = False
        for b in range(BLOC):
            mrow = stat_p.tile([1, NBLK], F32, tag="mrow")
            zrow = stat_p.tile([1, NBLK], F32, tag="zrow")
            ctxT_blks = []
            for blk in range(NBLK):
                nxt = (b, blk + 1) if blk + 1 < NBLK else (b + 1, 0)
                if (b, blk) in pre:
                    encT = pre.pop((b, blk))
                else:
                    encT = cur
                if nxt[0] < BLOC and nxt not in pre:
                    nxt_nat = load_enc(*nxt)
                    last_nat = nxt_nat
                else:
                    nxt_nat = None

                # enc_proj (k-tiles) + tanh -> e (bf16)
                e_sb = []
                for kt in range(NKT):
                    pse = ps_e.tile([128, SBLK], F32, tag="pe")
                    for ht in range(NHT):
                        nc.tensor.matmul(pse, wcT[:, ht, kt * 128:(kt + 1) * 128],
                                         encT[:, ht, :],
                                         start=(ht == 0), stop=(ht == NHT - 1))
                    et = e_p.tile([128, SBLK], BF16, tag="et")
                    nc.scalar.activation(et, pse, TT, bias=bias_kb[:, kt, b:b + 1])
                    e_sb.append(et)

                # next block's transposes BEFORE deferred work, so the encT
                # copy is never stuck behind softmax/combine in an engine FIFO
                if nxt_nat is not None:
                    cur = transpose_block(nxt_nat)

                # deferred non-PE tail of the previous block, then its combine
                if pending is not None:
                    wrow = do_scores_softmax(pending)
                    do_context(pending, wrow)
                    pending = None
                if pending_fin is not None:
                    do_combine(pending_fin)
                    pending_fin = None
                if b == BLOC - 1 and blk == 1 and not tails_emitted:
                    # batches 0..2 are combined by now; emit their row
                    # transposes here so they run as one burst during batch-3
                    # compute instead of stalling at the very end
                    identf_mid = consts.tile([128, 128], F32)
                    nc.vector.tensor_copy(identf_mid, identf)
                    for bb in range(BLOC - 1):
                        emit_tail(bb, accs[bb], identf_mid)
                    tails_emitted = True
                pending = (b, blk, encT, e_sb, mrow, zrow, ctxT_blks)
                if blk == NBLK - 1:
                    pending_fin = (b, mrow, zrow, ctxT_blks)

        # ---------------- final block: row-form context on PE ----------------
        b, blk, encT, e_sb, mrow, zrow, ctxT_blks = pending
        wrow = do_scores_softmax(pending)
        frow, fB = combine_factors(b, mrow, zrow)

        # acc012 = f0*c0 + f1*c1 + f2*c2 (column form; final block excluded)
        acc = acc_p.tile([128, NHT], F32, tag=f"acc{b}")
        nc.vector.tensor_scalar_mul(acc, ctxT_blks[0], fB[:, 0:1])
        for kblk in range(1, NBLK - 1):
            nc.vector.scalar_tensor_tensor(
                out=acc, in0=ctxT_blks[kblk], scalar=fB[:, kblk:kblk + 1],
                in1=acc, op0=MULT, op1=ADD)

        # w_scaled = wrow * f3; transpose to [s,1] cols; crow = w^T @ enc_nat
        wrow_s = row_p.tile([1, SBLK], BF16, tag="wrs")
        nc.vector.tensor_scalar_mul(wrow_s, wrow, frow[:, NBLK - 1:NBLK])
        psw = ps_tr.tile([128, SBLK], BF16, tag="tp")
        for st in range(NST):
            # bf16 PSUM writes must be 4-byte aligned: use even columns
            nc.tensor.transpose(psw[:, 2 * st:2 * st + 1],
                                wrow_s[:, st * 128:(st + 1) * 128],
                                identb[0:1, 0:1])
        wT = row_p.tile([128, NST], BF16, tag="wT")
        nc.vector.tensor_copy(wT, psw[:, 0:2 * NST:2])
        crow = stat_p.tile([1, H], F32, tag="crow")
        for hb in range(2):
            psc = ps_s.tile([1, SBLK], F32, tag="sc")
            for st in range(NST):
                nc.tensor.matmul(psc, wT[:, st:st + 1],
                                 last_nat[:, st, hb * 512:(hb + 1) * 512],
                                 start=(st == 0), stop=(st == NST - 1))
            nc.vector.tensor_copy(crow[:, hb * 512:(hb + 1) * 512], psc)

        # transpose acc012 to a row, add crow, store
        ctx_row = stat_p.tile([1, H], F32, tag="ctxr3")
        for hb in range(2):
            pso = ps_o.tile([1, 512], F32, tag="or")
            for hq in range(NHT // 2):
                ht = hb * (NHT // 2) + hq
                nc.tensor.transpose(pso[:, hq * 128:(hq + 1) * 128],
                                    acc[:, ht:ht + 1], identf)
            nc.vector.tensor_copy(ctx_row[:, hb * 512:(hb + 1) * 512], pso)
        yrow = stat_p.tile([1, H], F32, tag="yrow")
        nc.vector.tensor_tensor(out=yrow, in0=ctx_row, in1=crow, op=ADD)
        nc.sync.dma_start(out=y[BLOC - 1:BLOC, :], in_=yrow)

    nc.compile()
    return nc


_NC_CACHE = None


def _get_nc():
    global _NC_CACHE
    if _NC_CACHE is None:
        _NC_CACHE = build_kernel()
    return _NC_CACHE


def kernel(decoder_state, encoder_hiddens, Wa_w, Wa_b, Wb_w, Wb_b, Wc_w, Wc_b,
           **run_kwargs):
    decoder_state = np.ascontiguousarray(decoder_state, dtype=np.float32)
    encoder_hiddens = np.ascontiguousarray(encoder_hiddens, dtype=np.float32)
    nc = _get_nc()
    in_maps = []
    for c in range(NCORES):
        in_maps.append({
            "enc": encoder_hiddens[c * BLOC:(c + 1) * BLOC],
            "dec": decoder_state[c * BLOC:(c + 1) * BLOC],
            "wa": np.ascontiguousarray(Wa_w, dtype=np.float32).reshape(1, K),
            "wb": np.ascontiguousarray(Wb_w, dtype=np.float32),
            "wbb": np.ascontiguousarray(Wb_b, dtype=np.float32).reshape(1, K),
            "wc": np.ascontiguousarray(Wc_w, dtype=np.float32),
            "wcb": np.ascontiguousarray(Wc_b, dtype=np.float32).reshape(1, K),
        })
    res = bass_utils.run_bass_kernel_spmd(
        nc, in_maps, core_ids=list(range(NCORES)), **run_kwargs)
    out = np.concatenate([res.results[c]["y"] for c in range(NCORES)], axis=0)
    # Wa_b shifts every score equally; softmax is invariant to it.
    if run_kwargs:
        return out, res
    return out


# revision 14
# speedup vs baseline: 1.1410x; 1.0206x over previous
"""Bahdanau attention Trainium2 Bass kernel (bf16 PE pipeline).

Problem (fixed shapes):
  decoder_state [32, 1024] f32, encoder_hiddens [32, 2048, 1024] f32,
  Wa_w [1,1024], Wa_b [1], Wb_w [1024,1024], Wb_b [1024], Wc_w [1024,1024], Wc_b [1024]
  out: context [32, 1024] f32

Strategy: data-parallel over batch, 4 batches per core on 8 cores.
encoder_hiddens is cast f32->bf16 inside the SWDGE load; on-chip PE
transposes (bf16, one burst per block) produce [h,s] tiles; the PE streams
bf16 matmuls at the 1 col/cycle roofline (~216ns per 128x128x512 MM).
Scores run as PE matmuls deferred one block (et long ready -> no stall).
Softmax is flash-style per 512-wide s-block; context partials accumulate
on the DVE in [h,ht] column form; per-batch combines transpose to rows on
the PE. Emission order per section keeps each engine FIFO free of
head-of-line blocking: MMs+tanh, next-block transposes, then deferred
score/softmax/context work. The kernel-final block computes its context
directly in row form on the PE to shorten the serial tail.
"""
import sys

if "/opt/trn_rl_repo" not in sys.path:
    sys.path.insert(0, "/opt/trn_rl_repo")

import numpy as np

import concourse.bass as bass
import concourse.tile as tile
from concourse import bacc, mybir
from concourse import bass_utils
from concourse.masks import make_identity

F32 = mybir.dt.float32
F32R = mybir.dt.float32r
BF16 = mybir.dt.bfloat16

B, S, H, K = 32, 2048, 1024, 1024
NCORES = 8
BLOC = B // NCORES          # batches per core
SBLK = 512                  # s-block (softmax block, PE moving width)
NBLK = S // SBLK            # 4
NST = SBLK // 128           # s-tiles per block: 4
NHT = H // 128              # 8
NKT = K // 128              # 8


def build_kernel():
    nc = bacc.Bacc("TRN2", target_bir_lowering=False)

    enc = nc.dram_tensor("enc", [BLOC, S, H], F32, kind="ExternalInput")
    dec = nc.dram_tensor("dec", [BLOC, H], F32, kind="ExternalInput")
    wa = nc.dram_tensor("wa", [1, K], F32, kind="ExternalInput")
    wb = nc.dram_tensor("wb", [K, H], F32, kind="ExternalInput")
    wbb = nc.dram_tensor("wbb", [1, K], F32, kind="ExternalInput")
    wc = nc.dram_tensor("wc", [K, H], F32, kind="ExternalInput")
    wcb = nc.dram_tensor("wcb", [1, K], F32, kind="ExternalInput")
    y = nc.dram_tensor("y", [BLOC, H], F32, kind="ExternalOutput")

    TT = mybir.ActivationFunctionType.Tanh
    EX = mybir.ActivationFunctionType.Exp
    CP = mybir.ActivationFunctionType.Copy
    ADD = mybir.AluOpType.add
    MULT = mybir.AluOpType.mult

    from contextlib import ExitStack
    with tile.TileContext(nc) as tc, ExitStack() as stack:
        consts = stack.enter_context(tc.tile_pool(name="consts", bufs=1))
        identf = consts.tile([128, 128], F32)
        make_identity(nc, identf)
        identb = consts.tile([128, 128], BF16)
        nc.vector.tensor_copy(identb, identf)
        wcT = consts.tile([128, NHT, K], BF16)      # [h, ht, k]
        waT = consts.tile([128, NKT], BF16)
        bias_kb = consts.tile([128, NKT, BLOC], F32)

        # --- enc streaming pools ---
        enc_p = stack.enter_context(tc.tile_pool(name="enc_nat", bufs=2))
        encT_p = stack.enter_context(tc.tile_pool(name="encT", bufs=4))
        ps_tr = stack.enter_context(tc.tile_pool(name="ps_tr", bufs=3, space="PSUM"))

        def load_enc(b, blk):
            # SWDGE cast-load f32 -> bf16, [s_p=128, st, h]
            t = enc_p.tile([128, NST, H], BF16, tag="en")
            half = NST // 2
            for hh in range(2):
                nc.gpsimd.dma_start(
                    out=t[:, hh * half:(hh + 1) * half, :],
                    in_=enc[b, blk * SBLK + hh * half * 128:
                            blk * SBLK + (hh + 1) * half * 128, :].rearrange(
                        "(st sp) h -> sp st h", sp=128))
            return t

        def transpose_block(enc_nat):
            # PE transposes (bf16): [s, h] -> [h, s] tiles, one burst per block
            eT = encT_p.tile([128, NHT, SBLK], BF16, tag="eT")
            for ht in range(NHT):
                pst = ps_tr.tile([128, SBLK], BF16, tag="tp")
                for st in range(NST):
                    nc.tensor.transpose(pst[:, st * 128:(st + 1) * 128],
                                        enc_nat[:, st, ht * 128:(ht + 1) * 128],
                                        identb)
                nc.vector.tensor_copy(eT[:, ht, :], pst)
            return eT

        # ---------------- setup ----------------
        with tc.tile_pool(name="setup", bufs=1) as setup, \
             tc.tile_pool(name="setup_ps", bufs=1, space="PSUM") as sps:
            # SWDGE bf16 cast-loads, in dependency order: wc, wb, wa, enc
            wc_nat = setup.tile([128, NKT, H], BF16, tag="wc_nat")
            nc.gpsimd.dma_start(
                out=wc_nat, in_=wc.rearrange("(kt kp) h -> kp kt h", kp=128))
            pre_nat = {}
            pre_nat[(0, 0)] = load_enc(0, 0)
            wb_nat = setup.tile([128, NKT, H], BF16, tag="wb_nat")
            nc.gpsimd.dma_start(
                out=wb_nat, in_=wb.rearrange("(kt kp) h -> kp kt h", kp=128))
            wa_nat = setup.tile([1, K], BF16, tag="wa_nat")
            nc.gpsimd.dma_start(out=wa_nat, in_=wa[:, :])
            pre_nat[(0, 1)] = load_enc(0, 1)

            # HWDGE f32 loads (parallel queue-rings with SWDGE)
            dec_nat = setup.tile([BLOC, H], F32)
            nc.sync.dma_start(out=dec_nat, in_=dec[:, :])
            wbb_r = setup.tile([1, K], F32, tag="brin")
            wcb_r = setup.tile([1, K], F32, tag="brin2")
            nc.sync.dma_start(out=wbb_r, in_=wbb[:, :])
            nc.sync.dma_start(out=wcb_r, in_=wcb[:, :])

            # wcT via PE transposes (bf16), ps_tr ring
            for ht in range(NHT):
                for ktg in range(2):
                    psc = ps_tr.tile([128, SBLK], BF16, tag="tp")
                    for kq in range(NKT // 2):
                        kt = ktg * 4 + kq
                        nc.tensor.transpose(
                            psc[:, kq * 128:(kq + 1) * 128],
                            wc_nat[:, kt, ht * 128:(ht + 1) * 128], identb)
                    nc.scalar.activation(
                        wcT[:, ht, ktg * 512:(ktg + 1) * 512], psc, CP)

            # transpose the two prefetched enc blocks early
            pre = {}
            for bb in ((0, 0), (0, 1)):
                pre[bb] = transpose_block(pre_nat[bb])

            # wbT via PE transposes (bf16), ps_tr ring
            wbT = setup.tile([128, NHT, K], BF16, tag="wbT")   # [h, ht, k]
            for ht in range(NHT):
                for ktg in range(2):
                    psb = ps_tr.tile([128, SBLK], BF16, tag="tp")
                    for kq in range(NKT // 2):
                        kt = ktg * 4 + kq
                        nc.tensor.transpose(
                            psb[:, kq * 128:(kq + 1) * 128],
                            wb_nat[:, kt, ht * 128:(ht + 1) * 128], identb)
                    nc.scalar.activation(
                        wbT[:, ht, ktg * 512:(ktg + 1) * 512], psb, CP)

            # decT via PE transposes (f32), cast to bf16 for dec_proj MMs
            decT = setup.tile([128, NHT, BLOC], BF16)
            for ht in range(NHT):
                ps = sps.tile([128, BLOC], F32, tag="tp")
                nc.tensor.transpose(ps, dec_nat[:, ht * 128:(ht + 1) * 128],
                                    identf[0:BLOC, 0:BLOC])
                nc.vector.tensor_copy(decT[:, ht, :], ps)

            # waT via PE transposes (bf16)
            for kt in range(NKT):
                psw = sps.tile([128, 1], BF16, tag="tw")
                nc.tensor.transpose(psw, wa_nat[:, kt * 128:(kt + 1) * 128],
                                    identb[0:1, 0:1])
                nc.vector.tensor_copy(waT[:, kt:kt + 1], psw)

            # dec_proj[b, k] = sum_h decT[h,b].T @ wbT[h,k]
            dproj = setup.tile([BLOC, K], F32)
            for kh in range(2):
                psd = sps.tile([BLOC, 512], F32, tag="pd")
                for ht in range(NHT):
                    nc.tensor.matmul(psd, decT[:, ht, :],
                                     wbT[:, ht, kh * 512:(kh + 1) * 512],
                                     start=(ht == 0), stop=(ht == NHT - 1))
                nc.vector.tensor_copy(dproj[:, kh * 512:(kh + 1) * 512], psd)

            # bias row: Wb_b + Wc_b -> [k,1] segments
            brow = setup.tile([1, K], F32)
            nc.vector.tensor_tensor(out=brow, in0=wbb_r, in1=wcb_r, op=ADD)
            bseg = setup.tile([128, NKT], F32)
            for kt in range(NKT):
                ps1 = sps.tile([128, BLOC], F32, tag="tp")
                nc.tensor.transpose(ps1[:, 0:1], brow[:, kt * 128:(kt + 1) * 128],
                                    identf[0:1, 0:1])
                nc.vector.tensor_copy(bseg[:, kt:kt + 1], ps1[:, 0:1])

            # bias_kb[:, kt, b] = dproj^T + bseg
            for kt in range(NKT):
                ps2 = sps.tile([128, BLOC], F32, tag="tp")
                nc.tensor.transpose(ps2, dproj[:, kt * 128:(kt + 1) * 128],
                                    identf[0:BLOC, 0:BLOC])
                bs = bseg[:, kt:kt + 1]
                nc.vector.tensor_tensor(
                    out=bias_kb[:, kt, :], in0=ps2,
                    in1=bass.AP(tensor=bs.tensor, offset=bs.offset,
                                ap=[bs.ap[0], [0, BLOC]]),
                    op=ADD)

        # ---------------- main loop ----------------
        e_p = stack.enter_context(tc.tile_pool(name="e", bufs=18))
        row_p = stack.enter_context(tc.tile_pool(name="rows", bufs=3))
        stat_p = stack.enter_context(tc.tile_pool(name="stats", bufs=2))
        ctxT_p = stack.enter_context(tc.tile_pool(name="ctxT", bufs=10))
        acc_p = stack.enter_context(tc.tile_pool(name="acc", bufs=BLOC))
        bc_p = stack.enter_context(tc.tile_pool(name="bcast", bufs=3))
        ps_e = stack.enter_context(tc.tile_pool(name="ps_e", bufs=2, space="PSUM"))
        ps_s = stack.enter_context(tc.tile_pool(name="ps_s", bufs=2, space="PSUM"))
        ps_o = stack.enter_context(tc.tile_pool(name="ps_o", bufs=1, space="PSUM"))

        def do_scores_softmax(task):
            b, blk, encT, e_sb, mrow, zrow, ctxT_blks = task
            # scores: PE matmuls (deferred one block, et is long ready)
            pss = ps_s.tile([1, SBLK], F32, tag="sc")
            for kt in range(NKT):
                nc.tensor.matmul(pss, waT[:, kt:kt + 1], e_sb[kt],
                                 start=(kt == 0), stop=(kt == NKT - 1))

            # block softmax straight from PSUM: m_blk, w = exp(s-m) (bf16), Z
            negm = row_p.tile([1, 1], F32, tag="negm")
            nc.vector.reduce_max(negm, pss, axis=mybir.AxisListType.X,
                                 negate=True)
            wrow = row_p.tile([1, SBLK], BF16, tag="wrow")
            nc.scalar.activation(wrow, pss, EX, bias=negm,
                                 accum_out=zrow[:, blk:blk + 1])
            nc.vector.tensor_scalar_mul(mrow[:, blk:blk + 1], negm, -1.0)
            return wrow

        def do_context(task, wrow):
            b, blk, encT, e_sb, mrow, zrow, ctxT_blks = task
            # context partial on DVE (bf16 inputs, f32 accum)
            wb_t = bc_p.tile([128, SBLK], BF16, tag="wb")
            nc.gpsimd.partition_broadcast(wb_t, wrow, 128)
            ctxT_blk = ctxT_p.tile([128, NHT], F32, tag="ct")
            for ht in range(NHT):
                scr = bc_p.tile([128, SBLK], BF16, tag="scr")
                nc.vector.scalar_tensor_tensor(
                    out=scr, in0=encT[:, ht, :], scalar=1.0, in1=wb_t,
                    op0=MULT, op1=MULT, accum_out=ctxT_blk[:, ht:ht + 1])
            ctxT_blks.append(ctxT_blk)

        def combine_factors(b, mrow, zrow):
            # f[blk] = exp(m_blk - m_g) / Z, broadcast to all partitions
            negmg = stat_p.tile([1, 1], F32, tag="negmg")
            nc.vector.reduce_max(negmg, mrow, axis=mybir.AxisListType.X,
                                 negate=True)
            fb = stat_p.tile([1, NBLK], F32, tag="fb")
            nc.scalar.activation(fb, mrow, EX, bias=negmg)
            zf = stat_p.tile([1, NBLK], F32, tag="zf")
            nc.vector.tensor_tensor(out=zf, in0=zrow, in1=fb, op=MULT)
            z = stat_p.tile([1, 1], F32, tag="z")
            nc.vector.reduce_sum(z, zf, axis=mybir.AxisListType.X)
            rz = stat_p.tile([1, 1], F32, tag="rz")
            nc.vector.reciprocal(rz, z)
            frow = stat_p.tile([1, NBLK], F32, tag="frow")
            nc.vector.tensor_scalar_mul(frow, fb, rz)
            fB = stat_p.tile([128, NBLK], F32, tag="fB")
            nc.gpsimd.partition_broadcast(fB, frow, 128)
            return frow, fB

        def do_combine(task):
            # batch combine on DVE: acc = sum_blk C_blk * exp(m_blk - m_g) / Z
            b, mrow, zrow, ctxT_blks = task
            frow, fB = combine_factors(b, mrow, zrow)
            acc = acc_p.tile([128, NHT], F32, tag=f"acc{b}")
            nc.vector.tensor_scalar_mul(acc, ctxT_blks[0], fB[:, 0:1])
            for blk in range(1, NBLK):
                nc.vector.scalar_tensor_tensor(
                    out=acc, in0=ctxT_blks[blk], scalar=fB[:, blk:blk + 1],
                    in1=acc, op0=MULT, op1=ADD)
            accs.append(acc)

        def emit_tail(b, acc, ident):
            # transpose acc [h,ht] -> ctx_row [1, H], store
            ctx_row = stat_p.tile([1, H], F32, tag=f"ctxr{b % 2}")
            for hb in range(2):
                pso = ps_o.tile([1, 512], F32, tag="or")
                for hq in range(NHT // 2):
                    ht = hb * (NHT // 2) + hq
                    nc.tensor.transpose(pso[:, hq * 128:(hq + 1) * 128],
                                        acc[:, ht:ht + 1], ident)
                nc.vector.tensor_copy(ctx_row[:, hb * 512:(hb + 1) * 512], pso)
            nc.sync.dma_start(out=y[b:b + 1, :], in_=ctx_row)

        accs = []
        pending = None
        pending_fin = None
        cur = None
        last_nat = None
        tails_emitted = False
        for b in range(BLOC):
            mrow = stat_p.tile([1, NBLK], F32, tag="mrow")
            zrow = stat_p.tile([1, NBLK], F32, tag="zrow")
            ctxT_blks = []
            for blk in range(NBLK):
                nxt = (b, blk + 1) if blk + 1 < NBLK else (b + 1, 0)
                if (b, blk) in pre:
                    encT = pre.pop((b, blk))
                else:
                    encT = cur
                if nxt[0] < BLOC and nxt not in pre:
                    nxt_nat = load_enc(*nxt)
                    last_nat = nxt_nat
                else:
                    nxt_nat = None

                # enc_proj (k-tiles) + tanh -> e (bf16)
                e_sb = []
                for kt in range(NKT):
                    pse = ps_e.tile([128, SBLK], F32, tag="pe")
                    for ht in range(NHT):
                        nc.tensor.matmul(pse, wcT[:, ht, kt * 128:(kt + 1) * 128],
                                         encT[:, ht, :],
                                         start=(ht == 0), stop=(ht == NHT - 1))
                    et = e_p.tile([128, SBLK], BF16, tag="et")
                    nc.scalar.activation(et, pse, TT, bias=bias_kb[:, kt, b:b + 1])
                    e_sb.append(et)

                # next block's transposes BEFORE deferred work, so the encT
                # copy is never stuck behind softmax/combine in an engine FIFO
                if nxt_nat is not None:
                    cur = transpose_block(nxt_nat)

                # deferred non-PE tail of the previous block, then its combine
                if pending is not None:
                    wrow = do_scores_softmax(pending)
                    if b == BLOC - 1 and blk == NBLK - 1:
                        held_ctx = (pending, wrow)
                    else:
                        do_context(pending, wrow)
                    pending = None
                if pending_fin is not None:
                    do_combine(pending_fin)
                    pending_fin = None
                if b == BLOC - 1 and blk == 1 and not tails_emitted:
                    # batches 0..2 are combined by now; emit their row
                    # transposes here so they run as one burst during batch-3
                    # compute instead of stalling at the very end
                    identf_mid = consts.tile([128, 128], F32)
                    nc.vector.tensor_copy(identf_mid, identf)
                    for bb in range(BLOC - 1):
                        emit_tail(bb, accs[bb], identf_mid)
                    tails_emitted = True
                pending = (b, blk, encT, e_sb, mrow, zrow, ctxT_blks)
                if blk == NBLK - 1:
                    pending_fin = (b, mrow, zrow, ctxT_blks)

        # ---------------- final block: row-form context on PE ----------------
        b, blk, encT, e_sb, mrow, zrow, ctxT_blks = pending
        wrow = do_scores_softmax(pending)
        frow, fB = combine_factors(b, mrow, zrow)

        # w_scaled for the final block first: unblocks the PE row path
        wrow_s = row_p.tile([1, SBLK], BF16, tag="wrs")
        nc.vector.tensor_scalar_mul(wrow_s, wrow, frow[:, NBLK - 1:NBLK])

        # now the held (3,2) context STTs
        do_context(*held_ctx)

        # acc012 = f0*c0 + f1*c1 + f2*c2 (column form; final block excluded)
        acc = acc_p.tile([128, NHT], F32, tag=f"acc{b}")
        nc.vector.tensor_scalar_mul(acc, ctxT_blks[0], fB[:, 0:1])
        for kblk in range(1, NBLK - 1):
            nc.vector.scalar_tensor_tensor(
                out=acc, in0=ctxT_blks[kblk], scalar=fB[:, kblk:kblk + 1],
                in1=acc, op0=MULT, op1=ADD)

        # transpose w_scaled to [s,1] cols; crow = w^T @ enc_nat
        psw = ps_tr.tile([128, SBLK], BF16, tag="tp")
        for st in range(NST):
            # bf16 PSUM writes must be 4-byte aligned: use even columns
            nc.tensor.transpose(psw[:, 2 * st:2 * st + 1],
                                wrow_s[:, st * 128:(st + 1) * 128],
                                identb[0:1, 0:1])
        wT = row_p.tile([128, NST], BF16, tag="wT")
        nc.vector.tensor_copy(wT, psw[:, 0:2 * NST:2])
        crow = stat_p.tile([1, H], F32, tag="crow")
        for hb in range(2):
            psc = ps_s.tile([1, SBLK], F32, tag="sc")
            for st in range(NST):
                nc.tensor.matmul(psc, wT[:, st:st + 1],
                                 last_nat[:, st, hb * 512:(hb + 1) * 512],
                                 start=(st == 0), stop=(st == NST - 1))
            nc.vector.tensor_copy(crow[:, hb * 512:(hb + 1) * 512], psc)

        # transpose acc012 to a row, add crow, store
        ctx_row = stat_p.tile([1, H], F32, tag="ctxr3")
        for hb in range(2):
            pso = ps_o.tile([1, 512], F32, tag="or")
            for hq in range(NHT // 2):
                ht = hb * (NHT // 2) + hq
                nc.tensor.transpose(pso[:, hq * 128:(hq + 1) * 128],
                                    acc[:, ht:ht + 1], identf)
            nc.vector.tensor_copy(ctx_row[:, hb * 512:(hb + 1) * 512], pso)
        yrow = stat_p.tile([1, H], F32, tag="yrow")
        nc.vector.tensor_tensor(out=yrow, in0=ctx_row, in1=crow, op=ADD)
        nc.sync.dma_start(out=y[BLOC - 1:BLOC, :], in_=yrow)

    nc.compile()
    return nc


_NC_CACHE = None


def _get_nc():
    global _NC_CACHE
    if _NC_CACHE is None:
        _NC_CACHE = build_kernel()
    return _NC_CACHE


def kernel(decoder_state, encoder_hiddens, Wa_w, Wa_b, Wb_w, Wb_b, Wc_w, Wc_b,
           **run_kwargs):
    decoder_state = np.ascontiguousarray(decoder_state, dtype=np.float32)
    encoder_hiddens = np.ascontiguousarray(encoder_hiddens, dtype=np.float32)
    nc = _get_nc()
    in_maps = []
    for c in range(NCORES):
        in_maps.append({
            "enc": encoder_hiddens[c * BLOC:(c + 1) * BLOC],
            "dec": decoder_state[c * BLOC:(c + 1) * BLOC],
            "wa": np.ascontiguousarray(Wa_w, dtype=np.float32).reshape(1, K),
            "wb": np.ascontiguousarray(Wb_w, dtype=np.float32),
            "wbb": np.ascontiguousarray(Wb_b, dtype=np.float32).reshape(1, K),
            "wc": np.ascontiguousarray(Wc_w, dtype=np.float32),
            "wcb": np.ascontiguousarray(Wc_b, dtype=np.float32).reshape(1, K),
        })
    res = bass_utils.run_bass_kernel_spmd(
        nc, in_maps, core_ids=list(range(NCORES)), **run_kwargs)
    out = np.concatenate([res.results[c]["y"] for c in range(NCORES)], axis=0)
    # Wa_b shifts every score equally; softmax is invariant to it.
    if run_kwargs:
        return out, res
    return out
